# revision 26
# baseline (speedup 1.0000x reference)
"""AttentionSubsample on 8 Trainium2 NeuronCores — hand-written Bass/Tile kernel.

Strategy: data-parallel over batch B (64 -> 8 per core); weights replicated.
BatchNorm (training-mode, full-batch stats) is handled by computing raw
(un-normalized) projections plus per-channel sum / sum-of-squares on each
core, AllReducing the tiny stat vectors across the 8 cores, and folding the
resulting affine (scale, bias) into downstream consumers:

  * q-side: the full q affine, the k-channel scale and the 1/sqrt(KD) score
    scale fold into one per-channel affine applied to q^T in SBUF. The k-side
    BN *bias* adds a per-query constant to every score row, which softmax
    cancels, so k is consumed raw.
  * v-side: scale/bias commute through the attention average and fold into
    the post-attention eviction (out = (attn@v_raw)/S * s_v + t_v).
  * softmax: scores^T are built per 112-key chunk (K=32 contraction,
    4 heads packed into the 128 PE rows via tile_position), exponentiated on
    ScalarE straight out of PSUM, multiplied by the host-gathered
    exp(rel-pos-bias) table, and the denominator S falls out of the AV
    matmul via an extra all-ones column appended to v (M=65).
  * attention output is produced channel-major ([d, q]) so the proj matmul
    consumes it directly with no transpose; hard-swish runs in-between on
    full [128, 1568] tiles.

Everything heavy runs in bf16 on the TensorE (fp32 PSUM accumulation); the
stats, softmax denominators and final output stay fp32.

Host side: shards x over cores, pre-transposes to channel-major, splits W_kv
into k/v parts, gathers exp(attn_biases[:, bias_idxs]) (pure input prep) and
re-assembles the full [64, 196, 768] fp32 output from per-core [768, 1568]
channel-major shards.
"""

import numpy as np
import ml_dtypes

import concourse.bacc as bacc
import concourse.bass as bass
import concourse.tile as tile
import concourse.mybir as mybir
from concourse.bass_utils import run_bass_kernel_spmd

F32 = mybir.dt.float32
BF16 = mybir.dt.bfloat16
BF16NP = ml_dtypes.bfloat16
OP = mybir.AluOpType
AF = mybir.ActivationFunctionType

N_CORES = 8
RES, RES_ = 28, 14
H, KD, D = 16, 32, 64
C = 512
B = 64
BL = B // N_CORES            # 8 batches per core
N = RES * RES                # 784 keys per batch
NQ = RES_ * RES_             # 196 queries per batch
R = BL * N                   # 6272 kv rows per core
RQ = BL * NQ                 # 1568 q rows per core
KCH = H * KD                 # 512 k channels
VCH = H * D                  # 1024 v channels
PCH = 768                    # proj channels
KC = 112                     # key-chunk size (784 = 7*112)
NKC = N // KC                # 7
RT = 392                     # row-tile for projections (6272 = 16*392)
NRT = R // RT                # 16
NRTQ = RQ // RT              # 4
SCALE = float(KD) ** -0.5
EPS = 1e-5
NTOT_KV = float(B * N)       # 50176
NTOT_Q = float(B * NQ)       # 12544


DEBUG = False
NREP = 1
NO_CC = False


def _build():
    nc = bacc.Bacc("TRN2", target_bir_lowering=False, debug=False,
                   num_devices=N_CORES)
    dt = nc.dram_tensor
    xT_d = dt("xT", [C, R], BF16, kind="ExternalInput").ap()
    wk_d = dt("wkT", [C, KCH], BF16, kind="ExternalInput").ap()
    wv_d = dt("wvT", [C, VCH], BF16, kind="ExternalInput").ap()
    wq_d = dt("wqT", [C, KCH], BF16, kind="ExternalInput").ap()
    wp_d = dt("wpT", [VCH, PCH], BF16, kind="ExternalInput").ap()
    eb_d = dt("ebT", [KC, NKC, H, NQ], BF16, kind="ExternalInput").ap()
    gb_d = dt("gb", [128, 44], F32, kind="ExternalInput").ap()
    y_d = dt("y", [PCH, RQ], F32, kind="ExternalOutput").ap()
    dbg = None
    if DEBUG:
        dbg = {
            "dbg_kh": dt("dbg_kh", [C, R], BF16, kind="ExternalOutput").ap(),
            "dbg_q": dt("dbg_q", [C, RQ], BF16, kind="ExternalOutput").ap(),
            "dbg_ar1": dt("dbg_ar1", [128, 32], F32, kind="ExternalOutput").ap(),
            "dbg_aff": dt("dbg_aff", [128, 26], F32, kind="ExternalOutput").ap(),
            "dbg_hsw": dt("dbg_hsw", [VCH, RQ], BF16, kind="ExternalOutput").ap(),
            "dbg_attn": dt("dbg_attn", [128, NKC, 4, NQ], BF16, kind="ExternalOutput").ap(),
            "dbg_vaug": dt("dbg_vaug", [128, H * (D + 1)], BF16, kind="ExternalOutput").ap(),
            "dbg_av": dt("dbg_av", [D + 1, NQ], F32, kind="ExternalOutput").ap(),
        }

    with tile.TileContext(nc) as tc:
        _kernel(tc, y_d, xT_d, wk_d, wv_d, wq_d, wp_d, eb_d, gb_d, dbg)
    nc.compile()
    return nc


def _kernel(tc, y_d, xT_d, wk_d, wv_d, wq_d, wp_d, eb_d, gb_d, dbg=None):
    nc = tc.nc
    ctx_pools = {}

    # ---------------- persistent SBUF ----------------
    pers = tc.alloc_tile_pool(name="pers", bufs=1)
    gb_sb = pers.tile([128, 44], F32, tag="gb")
    kh_sb = pers.tile([128, 4, R], BF16, tag="kh")       # k^T raw
    q_sb = pers.tile([128, 4, RQ], BF16, tag="qt")       # q^T raw -> q~ after AR1
    hsw_sb = pers.tile([128, 8, RQ], BF16, tag="hsw")    # attnout^T -> hswish
    stat_sb = pers.tile([128, 32], F32, tag="st1")
    ar1_sb = pers.tile([128, 32], F32, tag="ar1")
    stat2_sb = pers.tile([128, 12], F32, tag="st2")
    ar2_sb = pers.tile([128, 12], F32, tag="ar2")
    aff_sb = pers.tile([128, 26], F32, tag="aff")  # qscale4 qbias4 sv8 tv8 sp6? -> see offsets
    # aff cols: 0:4 qscaleA, 4:8 qbiasA, 8:16 svA, 16:24 tvA  (proj affine in aff2)
    aff2_sb = pers.tile([128, 12], F32, tag="aff2")  # 0:6 spA, 6:12 tpA
    eps_sb = pers.tile([128, 1], F32, tag="eps")
    nc.vector.memset(eps_sb[:], EPS)
    dummy_sb = pers.tile([128, RT], F32, tag="dummy")
    nc.vector.memset(dummy_sb[:], 0.0)

    nc.sync.dma_start(gb_sb[:], gb_d[:])

    # DRAM scratch
    dram = tc.alloc_tile_pool(name="dram", bufs=1, space="DRAM")
    vT_dr = dram.tile([VCH, R + 16], BF16, tag="vT")
    yp_dr = dram.tile([PCH, RQ], F32, tag="yp")
    ar1_in = dram.tile([128, 32], F32, tag="ar1i")
    ar1_out = dram.tile([128, 32], F32, tag="ar1o")
    ar2_in = dram.tile([128, 12], F32, tag="ar2i")
    ar2_out = dram.tile([128, 12], F32, tag="ar2o")
    rcp_dr = dram.tile([BL, H, NQ], BF16, tag="rcp")

    for _rep in range(NREP):
        _phases(tc, nc, y_d, xT_d, wk_d, wv_d, wq_d, wp_d, eb_d, gb_d, dbg,
                pers, dram, gb_sb, kh_sb, q_sb, hsw_sb, stat_sb, ar1_sb,
                stat2_sb, ar2_sb, aff_sb, aff2_sb, eps_sb, dummy_sb,
                vT_dr, yp_dr, ar1_in, ar1_out, ar2_in, ar2_out, rcp_dr)

    pers.release()
    dram.release()


def _phases(tc, nc, y_d, xT_d, wk_d, wv_d, wq_d, wp_d, eb_d, gb_d, dbg,
            pers, dram, gb_sb, kh_sb, q_sb, hsw_sb, stat_sb, ar1_sb,
            stat2_sb, ar2_sb, aff_sb, aff2_sb, eps_sb, dummy_sb,
            vT_dr, yp_dr, ar1_in, ar1_out, ar2_in, ar2_out, rcp_dr):
    # ================= Phase 1: raw projections + stats =================
    with tc.tile_pool(name="p1sb", bufs=1) as p1sb, \
         tc.tile_pool(name="p1t", bufs=3) as p1t, \
         tc.tile_pool(name="p1ps", bufs=8, space="PSUM") as p1ps:
        wk_sb = p1sb.tile([128, 4, KCH], BF16, tag="wk")
        wv_sb = p1sb.tile([128, 4, VCH], BF16, tag="wv")
        wq_sb = p1sb.tile([128, 4, KCH], BF16, tag="wq")
        for t in range(4):
            nc.sync.dma_start(wk_sb[:, t, :], wk_d.rearrange("(t p) n -> p t n", p=128)[:, t, :])
            nc.sync.dma_start(wv_sb[:, t, :], wv_d.rearrange("(t p) n -> p t n", p=128)[:, t, :])
            nc.sync.dma_start(wq_sb[:, t, :], wq_d.rearrange("(t p) n -> p t n", p=128)[:, t, :])
        xT_sb = p1sb.tile([128, 4, R], BF16, tag="xT")
        for t in range(4):
            nc.sync.dma_start(
                xT_sb[:, t, :],
                xT_d.rearrange("(t p) n -> p t n", p=128)[:, t, :])

        ksum = p1sb.tile([128, 4, NRT], F32, tag="ksum")
        ksq = p1sb.tile([128, 4, NRT], F32, tag="ksq")
        vsum = p1sb.tile([128, 8, NRT], F32, tag="vsum")
        vsq = p1sb.tile([128, 8, NRT], F32, tag="vsq")
        qsum = p1sb.tile([128, 4, NRTQ], F32, tag="qsum")
        qsq = p1sb.tile([128, 4, NRTQ], F32, tag="qsq")

        def proj_tile(dst_slice, w_sb, ct, rt, rhs_fn, sumt, sqt, nrt_cols):
            ps = p1ps.tile([128, RT], F32, tag="ps")
            for cc in range(4):
                nc.tensor.matmul(
                    ps[:], w_sb[:, cc, ct * 128:(ct + 1) * 128], rhs_fn(cc),
                    start=(cc == 0), stop=(cc == 3))
            # fused eviction + per-channel row-sum
            nc.vector.scalar_tensor_tensor(
                out=dst_slice, in0=ps[:], scalar=1.0, in1=dummy_sb[:],
                op0=OP.mult, op1=OP.bypass, accum_out=sumt[:, ct, rt:rt + 1])
            junk = p1t.tile([128, RT], BF16, tag="junk")
            nc.vector.scalar_tensor_tensor(
                out=junk[:], in0=dst_slice, scalar=1.0, in1=dst_slice,
                op0=OP.mult, op1=OP.mult, accum_out=sqt[:, ct, rt:rt + 1])

        # --- v^T (to DRAM) ---
        for ct in range(8):
            for rt in range(NRT):
                ps = p1ps.tile([128, RT], F32, tag="ps")
                for cc in range(4):
                    nc.tensor.matmul(
                        ps[:], wv_sb[:, cc, ct * 128:(ct + 1) * 128],
                        xT_sb[:, cc, rt * RT:(rt + 1) * RT],
                        start=(cc == 0), stop=(cc == 3))
                vtile = p1t.tile([128, RT], BF16, tag="vtile")
                nc.scalar.activation(
                    vtile[:], ps[:], AF.Identity,
                    bias=dummy_sb[:, 0:1], scale=1.0,
                    accum_out=vsum[:, ct, rt:rt + 1])
                junk = p1t.tile([128, RT], BF16, tag="junk")
                nc.vector.scalar_tensor_tensor(
                    out=junk[:], in0=vtile[:], scalar=1.0, in1=vtile[:],
                    op0=OP.mult, op1=OP.mult,
                    accum_out=vsq[:, ct, rt:rt + 1])
                nc.sync.dma_start(
                    vT_dr.rearrange("(t p) n -> p t n", p=128)[:, ct,
                                                              rt * RT:(rt + 1) * RT],
                    vtile[:])

        # --- k^T (stays in SBUF) ---
        for ct in range(4):
            for rt in range(NRT):
                proj_tile(kh_sb[:, ct, rt * RT:(rt + 1) * RT], wk_sb, ct, rt,
                          lambda cc: xT_sb[:, cc, rt * RT:(rt + 1) * RT],
                          ksum, ksq, NRT)

        # --- q^T (subsampled rows; stays in SBUF) ---
        for ct in range(4):
            for rt in range(NRTQ):
                b0 = rt * 2  # 392 q-rows = 2 batches
                def qrhs(cc, b0=b0):
                    base = xT_sb[:, cc, :].rearrange(
                        "p (b i ti j tj) -> p b i ti j tj",
                        b=BL, i=RES_, ti=2, j=RES_, tj=2)
                    return base[:, b0:b0 + 2, :, 0, :, 0]
                proj_tile(q_sb[:, ct, rt * RT:(rt + 1) * RT], wq_sb, ct, rt,
                          qrhs, qsum, qsq, NRTQ)

        # --- fold stats into AR block ---
        nc.vector.tensor_reduce(stat_sb[:, 0:4], ksum[:], axis=mybir.AxisListType.X, op=OP.add)
        nc.vector.tensor_reduce(stat_sb[:, 4:8], ksq[:], axis=mybir.AxisListType.X, op=OP.add)
        nc.vector.tensor_reduce(stat_sb[:, 8:12], qsum[:], axis=mybir.AxisListType.X, op=OP.add)
        nc.vector.tensor_reduce(stat_sb[:, 12:16], qsq[:], axis=mybir.AxisListType.X, op=OP.add)
        nc.vector.tensor_reduce(stat_sb[:, 16:24], vsum[:], axis=mybir.AxisListType.X, op=OP.add)
        nc.vector.tensor_reduce(stat_sb[:, 24:32], vsq[:], axis=mybir.AxisListType.X, op=OP.add)

    # ---------------- AllReduce #1 ----------------
    nc.sync.dma_start(ar1_in[:], stat_sb[:])
    if NO_CC:
        nc.sync.dma_start(ar1_out[:], ar1_in[:])
    else:
        nc.gpsimd.collective_compute(
            "AllReduce", OP.add, replica_groups=[list(range(N_CORES))],
            ins=[ar1_in.opt()], outs=[ar1_out.opt()])
    nc.sync.dma_start(ar1_sb[:], ar1_out[:])

    # ---------------- post-AR1 affine math (tiny) ----------------
    with tc.tile_pool(name="aff_t", bufs=1) as afft:
        tmp = afft.tile([128, 16], F32, tag="t0")
        # k: cols 0:4 mean, 4:8 E[x^2]
        mk = tmp[:, 0:4]
        nc.vector.tensor_scalar_mul(mk, ar1_sb[:, 0:4], 1.0 / NTOT_KV)
        vk = tmp[:, 4:8]
        nc.vector.tensor_scalar_mul(vk, ar1_sb[:, 4:8], 1.0 / NTOT_KV)
        mk2 = tmp[:, 8:12]
        nc.vector.tensor_mul(mk2, mk, mk)
        nc.vector.tensor_sub(vk, vk, mk2)
        sdk = tmp[:, 8:12]
        nc.scalar.activation(sdk, vk, AF.Sqrt, bias=eps_sb[:])
        rsk = tmp[:, 4:8]
        nc.vector.reciprocal(rsk, sdk)
        skf = tmp[:, 8:12]  # full k scale = g_k / sd
        nc.vector.tensor_mul(skf, gb_sb[:, 0:4], rsk)

        # q: ar cols 8:12 sum, 12:16 sumsq ; gb cols 8:12 gq, 12:16 bq
        tq = afft.tile([128, 16], F32, tag="t1")
        mq = tq[:, 0:4]
        nc.vector.tensor_scalar_mul(mq, ar1_sb[:, 8:12], 1.0 / NTOT_Q)
        vq = tq[:, 4:8]
        nc.vector.tensor_scalar_mul(vq, ar1_sb[:, 12:16], 1.0 / NTOT_Q)
        mq2 = tq[:, 8:12]
        nc.vector.tensor_mul(mq2, mq, mq)
        nc.vector.tensor_sub(vq, vq, mq2)
        sdq = tq[:, 8:12]
        nc.scalar.activation(sdq, vq, AF.Sqrt, bias=eps_sb[:])
        rsq = tq[:, 4:8]
        nc.vector.reciprocal(rsq, sdq)
        sqf = tq[:, 8:12]
        nc.vector.tensor_mul(sqf, gb_sb[:, 8:12], rsq)
        # qscaleA = sqf * skf * SCALE ; qbiasA = (bq - mq*sqf) * skf * SCALE
        nc.vector.tensor_mul(aff_sb[:, 0:4], sqf, skf)
        nc.vector.tensor_scalar_mul(aff_sb[:, 0:4], aff_sb[:, 0:4], SCALE)
        tq2 = tq[:, 12:16]
        nc.vector.tensor_mul(tq2, mq, sqf)
        nc.vector.tensor_sub(tq2, gb_sb[:, 12:16], tq2)
        nc.vector.tensor_mul(aff_sb[:, 4:8], tq2, skf)
        nc.vector.tensor_scalar_mul(aff_sb[:, 4:8], aff_sb[:, 4:8], SCALE)

        # v: ar cols 16:24 sum, 24:32 sumsq; gb 16:24 gv, 24:32 bv
        tv = afft.tile([128, 24], F32, tag="t2")
        mv = tv[:, 0:8]
        nc.vector.tensor_scalar_mul(mv, ar1_sb[:, 16:24], 1.0 / NTOT_KV)
        vv = tv[:, 8:16]
        nc.vector.tensor_scalar_mul(vv, ar1_sb[:, 24:32], 1.0 / NTOT_KV)
        mv2 = tv[:, 16:24]
        nc.vector.tensor_mul(mv2, mv, mv)
        nc.vector.tensor_sub(vv, vv, mv2)
        sdv = tv[:, 16:24]
        nc.scalar.activation(sdv, vv, AF.Sqrt, bias=eps_sb[:])
        rsv = tv[:, 8:16]
        nc.vector.reciprocal(rsv, sdv)
        nc.vector.tensor_mul(aff_sb[:, 8:16], gb_sb[:, 16:24], rsv)   # svA
        nc.vector.tensor_mul(tv[:, 16:24], mv, aff_sb[:, 8:16])
        nc.vector.tensor_sub(aff_sb[:, 16:24], gb_sb[:, 24:32], tv[:, 16:24])  # tvA

    # q~ = q_raw * qscaleA + qbiasA (in place)
    for ct in range(4):
        nc.scalar.activation(q_sb[:, ct, :], q_sb[:, ct, :], AF.Identity,
                             bias=aff_sb[:, 4 + ct:5 + ct],
                             scale=aff_sb[:, ct:ct + 1])

    # ================= Phase 2: attention =================
    with tc.tile_pool(name="ebp", bufs=1) as ebp, \
         tc.tile_pool(name="vstream", bufs=8) as vstream, \
         tc.tile_pool(name="attnp", bufs=3) as attnp, \
         tc.tile_pool(name="smallp", bufs=5) as smallp, \
         tc.tile_pool(name="qblkp", bufs=2) as qblkp, \
         tc.tile_pool(name="qkps", bufs=2, space="PSUM") as qkps, \
         tc.tile_pool(name="avps", bufs=4, space="PSUM") as avps:
        eb_sb = ebp.tile([128, NKC, H, NQ], BF16, tag="eb")
        nc.sync.dma_start(eb_sb[0:KC, :, :, :], eb_d[:])

        for b in range(BL):
            vts = []
            for kc in range(NKC):
                vtmp = vstream.tile([128, VCH], BF16, tag="vtmp")
                nc.sync.dma_start_transpose(
                    vtmp[:], vT_dr[:, b * N + kc * KC: b * N + kc * KC + 128])
                vt = vstream.tile([128, H * (D + 1)], BF16, tag="vaug")
                nc.gpsimd.tensor_copy(
                    vt.rearrange("p (h d) -> p h d", h=H)[:, :, 0:D],
                    vtmp.rearrange("p (h d) -> p h d", h=H)[:, :, :])
                nc.vector.memset(
                    vt.rearrange("p (h d) -> p h d", h=H)[0:KC, :, D:D + 1],
                    1.0)
                if dbg is not None and b == 0 and kc == 0:
                    nc.sync.dma_start(dbg["dbg_vaug"][:], vt[:])
                vts.append(vt)

            for quad in range(4):
                attn = attnp.tile([128, NKC, 4, NQ], BF16, tag="attn")
                # block-sparse packed q: rows hq*32..+32 hold head hq's q at
                # cols hq*196..+196, zeros elsewhere -> one [128,112] LDW per
                # (quad, kc) instead of four, K=128 contraction.
                qblk = qblkp.tile([128, 4, NQ], BF16, tag="qblk")
                nc.vector.memset(qblk[:], 0.0)
                for hq in range(4):
                    nc.gpsimd.tensor_copy(
                        qblk[hq * 32:(hq + 1) * 32, hq, :],
                        q_sb[hq * 32:(hq + 1) * 32, quad,
                             b * NQ:(b + 1) * NQ])
                for kc in range(NKC):
                    qk = qkps.tile([128, 2, 512], F32, tag="qk")
                    for j in range(2):
                        nc.tensor.matmul(
                            qk[0:KC, j, 0:2 * NQ],
                            kh_sb[:, quad,
                                  b * N + kc * KC: b * N + (kc + 1) * KC],
                            qblk[:, 2 * j:2 * j + 2, :],
                            start=True, stop=True)
                    attn_flat = attn[:].rearrange("p a b c -> p a (b c)")
                    nc.scalar.activation(
                        attn_flat[0:KC, kc, :].rearrange(
                            "p (j x) -> p j x", j=2),
                        qk[0:KC, :, 0:2 * NQ], AF.Exp)
                # multiply by exp(rel-pos bias), in place
                nc.vector.tensor_mul(
                    attn[0:KC, :, :, :],
                    attn[0:KC, :, :, :],
                    eb_sb[0:KC, :, quad * 4:(quad + 1) * 4, :])
                if dbg is not None and b == 0 and quad == 0:
                    nc.sync.dma_start(dbg["dbg_attn"][:], attn[:])

                for hq in range(4):
                    h = quad * 4 + hq
                    av = avps.tile([D + 1, NQ], F32, tag="av")
                    for kc in range(NKC):
                        nc.tensor.matmul(
                            av[:], vts[kc][0:KC, h * (D + 1):(h + 1) * (D + 1)],
                            attn[0:KC, kc, hq, :],
                            start=(kc == 0), stop=(kc == NKC - 1))
                    if dbg is not None and b == 0 and quad == 0 and hq == 0:
                        avdbg = smallp.tile([D + 1, NQ], F32, tag="avdbg")
                        nc.vector.tensor_copy(avdbg[:], av[:])
                        nc.sync.dma_start(dbg["dbg_av"][:], avdbg[:])
                    rrow = smallp.tile([1, NQ], BF16, tag="rrow")
                    rrow32 = smallp.tile([1, NQ], F32, tag="rrow32")
                    nc.vector.reciprocal(rrow32[:], av[D:D + 1, :])
                    nc.vector.tensor_copy(rrow[:], rrow32[:])
                    nc.sync.dma_start(rcp_dr[b, h, :], rrow[:])
                    rbc = smallp.tile([64, NQ], BF16, tag="rbc")
                    rap = rcp_dr[b, h, :]
                    nc.gpsimd.dma_start(
                        rbc[:],
                        bass.AP(tensor=rap.tensor, offset=rap.offset,
                                ap=[[0, 64]] + list(rap.ap)))
                    nc.vector.tensor_mul(
                        hsw_sb[(h % 2) * 64:(h % 2) * 64 + 64, h // 2,
                               b * NQ:(b + 1) * NQ],
                        av[0:D, :], rbc[:])

    # ---------------- hard-swish (with v affine) ----------------
    with tc.tile_pool(name="hswp", bufs=2) as hswp:
        for t in range(8):
            u = hsw_sb[:, t, :]
            nc.scalar.activation(u, u, AF.Identity,
                                 bias=aff_sb[:, 16 + t:17 + t],
                                 scale=aff_sb[:, 8 + t:9 + t])
            z = hswp.tile([128, RQ], BF16, tag="z")
            nc.vector.tensor_scalar(out=z[:], in0=u, scalar1=3.0, scalar2=0.0,
                                    op0=OP.add, op1=OP.max)
            nc.vector.tensor_scalar(out=z[:], in0=z[:], scalar1=6.0,
                                    scalar2=1.0 / 6.0, op0=OP.min, op1=OP.mult)
            nc.vector.tensor_mul(u, u, z[:])

    if dbg is not None:
        for t in range(4):
            nc.sync.dma_start(
                dbg["dbg_kh"].rearrange("(t p) n -> p t n", p=128)[:, t, :],
                kh_sb[:, t, :])
            nc.sync.dma_start(
                dbg["dbg_q"].rearrange("(t p) n -> p t n", p=128)[:, t, :],
                q_sb[:, t, :])
        for t in range(8):
            nc.sync.dma_start(
                dbg["dbg_hsw"].rearrange("(t p) n -> p t n", p=128)[:, t, :],
                hsw_sb[:, t, :])
        nc.sync.dma_start(dbg["dbg_ar1"][:], ar1_sb[:])
        nc.sync.dma_start(dbg["dbg_aff"][:], aff_sb[:])

    # ================= Phase 3: proj + BN =================
    with tc.tile_pool(name="p3w", bufs=1) as p3w, \
         tc.tile_pool(name="p3t", bufs=4) as p3t, \
         tc.tile_pool(name="p3ps", bufs=4, space="PSUM") as p3ps:
        wp_sb = p3w.tile([128, 8, PCH], BF16, tag="wp")
        for t in range(8):
            nc.sync.dma_start(wp_sb[:, t, :], wp_d.rearrange("(t p) n -> p t n", p=128)[:, t, :])
        psum_acc = p3t.tile([128, 6, NRTQ], F32, tag="psum_acc")
        psq_acc = p3t.tile([128, 6, NRTQ], F32, tag="psq_acc")
        for pt in range(6):
            for rt in range(NRTQ):
                ps = p3ps.tile([128, RT], F32, tag="pps")
                for cc in range(8):
                    nc.tensor.matmul(
                        ps[:], wp_sb[:, cc, pt * 128:(pt + 1) * 128],
                        hsw_sb[:, cc, rt * RT:(rt + 1) * RT],
                        start=(cc == 0), stop=(cc == 7))
                yb = p3t.tile([128, RT], F32, tag="yb")
                nc.vector.scalar_tensor_tensor(
                    out=yb[:], in0=ps[:], scalar=1.0, in1=dummy_sb[:],
                    op0=OP.mult, op1=OP.bypass,
                    accum_out=psum_acc[:, pt, rt:rt + 1])
                junk = p3t.tile([128, RT], F32, tag="junk3")
                nc.vector.scalar_tensor_tensor(
                    out=junk[:], in0=yb[:], scalar=1.0, in1=yb[:],
                    op0=OP.mult, op1=OP.mult,
                    accum_out=psq_acc[:, pt, rt:rt + 1])
                nc.sync.dma_start(
                    yp_dr.rearrange("(t p) n -> p t n", p=128)[:, pt,
                                                               rt * RT:(rt + 1) * RT],
                    yb[:])
        nc.vector.tensor_reduce(stat2_sb[:, 0:6], psum_acc[:],
                                axis=mybir.AxisListType.X, op=OP.add)
        nc.vector.tensor_reduce(stat2_sb[:, 6:12], psq_acc[:],
                                axis=mybir.AxisListType.X, op=OP.add)

    nc.sync.dma_start(ar2_in[:], stat2_sb[:])
    if NO_CC:
        nc.sync.dma_start(ar2_out[:], ar2_in[:])
    else:
        nc.gpsimd.collective_compute(
            "AllReduce", OP.add, replica_groups=[list(range(N_CORES))],
            ins=[ar2_in.opt()], outs=[ar2_out.opt()])
    nc.sync.dma_start(ar2_sb[:], ar2_out[:])

    with tc.tile_pool(name="finp", bufs=3) as finp, \
         tc.tile_pool(name="fint", bufs=1) as fint:
        tp = fint.tile([128, 18], F32, tag="tp")
        mp = tp[:, 0:6]
        nc.vector.tensor_scalar_mul(mp, ar2_sb[:, 0:6], 1.0 / NTOT_Q)
        vp = tp[:, 6:12]
        nc.vector.tensor_scalar_mul(vp, ar2_sb[:, 6:12], 1.0 / NTOT_Q)
        mp2 = tp[:, 12:18]
        nc.vector.tensor_mul(mp2, mp, mp)
        nc.vector.tensor_sub(vp, vp, mp2)
        sdp = tp[:, 12:18]
        nc.scalar.activation(sdp, vp, AF.Sqrt, bias=eps_sb[:])
        rsp = tp[:, 6:12]
        nc.vector.reciprocal(rsp, sdp)
        nc.vector.tensor_mul(aff2_sb[:, 0:6], gb_sb[:, 32:38], rsp)      # spA
        nc.vector.tensor_mul(tp[:, 12:18], mp, aff2_sb[:, 0:6])
        nc.vector.tensor_sub(aff2_sb[:, 6:12], gb_sb[:, 38:44], tp[:, 12:18])  # tpA

        for pt in range(6):
            yt = finp.tile([128, RQ], F32, tag="yt")
            nc.sync.dma_start(
                yt[:], yp_dr.rearrange("(t p) n -> p t n", p=128)[:, pt, :])
            nc.scalar.activation(yt[:], yt[:], AF.Identity,
                                 bias=aff2_sb[:, 6 + pt:7 + pt],
                                 scale=aff2_sb[:, pt:pt + 1])
            nc.sync.dma_start(
                y_d.rearrange("(t p) n -> p t n", p=128)[:, pt, :], yt[:])


# ==================== host staging ====================

_K_IDX = np.array([h * (KD + D) + j for h in range(H) for j in range(KD)])
_V_IDX = np.array([h * (KD + D) + KD + j for h in range(H) for j in range(D)])


def _stage(inputs):
    """Full inputs -> (shared_map, list of per-core xT)."""
    x = np.asarray(inputs["x"], np.float32)
    W_kv = np.asarray(inputs["W_kv"], np.float32)
    g_kv = np.asarray(inputs["g_kv"], np.float32)
    b_kv = np.asarray(inputs["b_kv"], np.float32)
    W_q = np.asarray(inputs["W_q"], np.float32)
    W_p = np.asarray(inputs["W_proj"], np.float32)
    ab = np.asarray(inputs["attn_biases"], np.float32)
    bi = np.asarray(inputs["bias_idxs"])

    shared = {}
    shared["wkT"] = np.ascontiguousarray(W_kv[_K_IDX].T).astype(BF16NP)
    shared["wvT"] = np.ascontiguousarray(W_kv[_V_IDX].T).astype(BF16NP)
    shared["wqT"] = np.ascontiguousarray(W_q.T).astype(BF16NP)
    shared["wpT"] = np.ascontiguousarray(W_p.T).astype(BF16NP)

    eb = np.exp(ab[:, bi])                     # [16, 196, 784]
    ebT = eb.transpose(0, 2, 1).reshape(H, NKC, KC, NQ).transpose(2, 1, 0, 3)
    shared["ebT"] = np.ascontiguousarray(ebT).astype(BF16NP)

    gb = np.zeros((128, 44), np.float32)
    gb[:, 0:4] = g_kv[_K_IDX].reshape(4, 128).T
    gb[:, 4:8] = b_kv[_K_IDX].reshape(4, 128).T
    gb[:, 8:12] = np.asarray(inputs["g_q"], np.float32).reshape(4, 128).T
    gb[:, 12:16] = np.asarray(inputs["b_q"], np.float32).reshape(4, 128).T
    gb[:, 16:24] = g_kv[_V_IDX].reshape(8, 128).T
    gb[:, 24:32] = b_kv[_V_IDX].reshape(8, 128).T
    gb[:, 32:38] = np.asarray(inputs["g_proj"], np.float32).reshape(6, 128).T
    gb[:, 38:44] = np.asarray(inputs["b_proj"], np.float32).reshape(6, 128).T
    shared["gb"] = gb

    xts = []
    for c in range(N_CORES):
        xl = x[c * BL:(c + 1) * BL]                      # [8, 784, 512]
        xts.append(np.ascontiguousarray(
            xl.transpose(2, 0, 1).reshape(C, R)).astype(BF16NP))
    return shared, xts


_nc = None


def _get_nc():
    global _nc
    if _nc is None:
        _nc = _build()
    return _nc


def kernel(**inputs):
    import jax
    dargs = _device_args(inputs)
    outs = run_on_device(dargs)
    jax.block_until_ready(outs)
    _, in_names, out_names, out_avals, _ = _get_jit()
    yi = out_names.index("y")
    yp_all = np.asarray(outs[yi]).reshape(N_CORES, PCH, RQ)
    out = np.empty((B, NQ, PCH), np.float32)
    for c in range(N_CORES):
        out[c * BL:(c + 1) * BL] = yp_all[c].T.reshape(BL, NQ, PCH)
    return out


# -------- device-resident timing protocol (mirrors previous baseline) --------

_jit_state = None


def _get_jit():
    """Build (once) a cached jitted shard_map executor for the NEFF."""
    global _jit_state
    if _jit_state is not None:
        return _jit_state
    import jax
    from jax.sharding import Mesh, PartitionSpec
    from jax.experimental.shard_map import shard_map
    from concourse import bass2jax, mybir as _mb

    nc = _get_nc()
    bass2jax.install_neuronx_cc_hook()
    partition_name = (nc.partition_id_tensor.name
                      if nc.partition_id_tensor else None)
    in_names, out_names, out_avals = [], [], []
    for alloc in nc.m.functions[0].allocations:
        if not isinstance(alloc, _mb.MemoryLocationSet):
            continue
        name = alloc.memorylocations[0].name
        if alloc.kind == "ExternalInput":
            if name != partition_name:
                in_names.append(name)
        elif alloc.kind == "ExternalOutput":
            out_names.append(name)
            out_avals.append(jax.core.ShapedArray(
                tuple(alloc.tensor_shape), _mb.dt.np(alloc.dtype)))
    n_params = len(in_names)
    all_in = in_names + out_names
    if partition_name is not None:
        all_in = all_in + [partition_name]

    def _body(*args):
        operands = list(args)
        if partition_name is not None:
            operands.append(bass2jax.partition_id_tensor())
        outs = bass2jax._bass_exec_p.bind(
            *operands, out_avals=tuple(out_avals),
            in_names=tuple(all_in), out_names=tuple(out_names),
            lowering_input_output_aliases=(),
            sim_require_finite=True, sim_require_nnan=True, nc=nc)
        return tuple(outs)

    devices = jax.devices()[:N_CORES]
    mesh = Mesh(np.asarray(devices), ("core",))
    n_outs = len(out_names)
    sharded = jax.jit(shard_map(
        _body, mesh=mesh,
        in_specs=(PartitionSpec("core"),) * (n_params + n_outs),
        out_specs=(PartitionSpec("core"),) * n_outs,
        check_rep=False), keep_unused=True)
    _jit_state = (sharded, in_names, out_names, out_avals, mesh)
    return _jit_state


def _device_args(inputs):
    import jax
    from jax.sharding import NamedSharding, PartitionSpec
    sharded, in_names, out_names, out_avals, mesh = _get_jit()
    sh = NamedSharding(mesh, PartitionSpec("core"))
    shared, xts = _stage(inputs)
    per_core = [{**shared, "xT": xts[c]} for c in range(N_CORES)]
    concat = [np.concatenate([np.asarray(per_core[c][n])
                              for c in range(N_CORES)], axis=0)
              for n in in_names]
    zeros = [np.zeros((N_CORES * a.shape[0], *a.shape[1:]), a.dtype)
             for a in out_avals]
    return tuple(jax.device_put(a, sh) for a in (*concat, *zeros))


def run_on_device(dargs):
    sharded, *_ = _get_jit()
    return sharded(*dargs)


if __name__ == "__main__":
    import reference
    inputs = {k: np.asarray(v) for k, v in reference.setup_inputs().items()}
    expected = np.asarray(reference.reference(**inputs))
    actual = kernel(**inputs)
    err = np.linalg.norm(actual - expected) / np.linalg.norm(expected)
    print("Relative error:", err)


# revision 27
# speedup vs baseline: 1.0299x; 1.0299x over previous
"""AttentionSubsample on 8 Trainium2 NeuronCores — hand-written Bass/Tile kernel.

Strategy: data-parallel over batch B (64 -> 8 per core); weights replicated.
BatchNorm (training-mode, full-batch stats) is handled by computing raw
(un-normalized) projections plus per-channel sum / sum-of-squares on each
core, AllReducing the tiny stat vectors across the 8 cores, and folding the
resulting affine (scale, bias) into downstream consumers:

  * q-side: the full q affine, the k-channel scale and the 1/sqrt(KD) score
    scale fold into one per-channel affine applied to q^T in SBUF. The k-side
    BN *bias* adds a per-query constant to every score row, which softmax
    cancels, so k is consumed raw.
  * v-side: scale/bias commute through the attention average and fold into
    the post-attention eviction (out = (attn@v_raw)/S * s_v + t_v).
  * softmax: scores^T are built per 112-key chunk (K=32 contraction,
    4 heads packed into the 128 PE rows via tile_position), exponentiated on
    ScalarE straight out of PSUM, multiplied by the host-gathered
    exp(rel-pos-bias) table, and the denominator S falls out of the AV
    matmul via an extra all-ones column appended to v (M=65).
  * attention output is produced channel-major ([d, q]) so the proj matmul
    consumes it directly with no transpose; hard-swish runs in-between on
    full [128, 1568] tiles.

Everything heavy runs in bf16 on the TensorE (fp32 PSUM accumulation); the
stats, softmax denominators and final output stay fp32.

Host side: shards x over cores, pre-transposes to channel-major, splits W_kv
into k/v parts, gathers exp(attn_biases[:, bias_idxs]) (pure input prep) and
re-assembles the full [64, 196, 768] fp32 output from per-core [768, 1568]
channel-major shards.
"""

import numpy as np
import ml_dtypes

import concourse.bacc as bacc
import concourse.bass as bass
import concourse.tile as tile
import concourse.mybir as mybir
from concourse.bass_utils import run_bass_kernel_spmd

F32 = mybir.dt.float32
BF16 = mybir.dt.bfloat16
BF16NP = ml_dtypes.bfloat16
OP = mybir.AluOpType
AF = mybir.ActivationFunctionType

N_CORES = 8
RES, RES_ = 28, 14
H, KD, D = 16, 32, 64
C = 512
B = 64
BL = B // N_CORES            # 8 batches per core
N = RES * RES                # 784 keys per batch
NQ = RES_ * RES_             # 196 queries per batch
R = BL * N                   # 6272 kv rows per core
RQ = BL * NQ                 # 1568 q rows per core
KCH = H * KD                 # 512 k channels
VCH = H * D                  # 1024 v channels
PCH = 768                    # proj channels
KC = 112                     # key-chunk size (784 = 7*112)
NKC = N // KC                # 7
RT = 392                     # row-tile for projections (6272 = 16*392)
NRT = R // RT                # 16
NRTQ = RQ // RT              # 4
SCALE = float(KD) ** -0.5
EPS = 1e-5
NTOT_KV = float(B * N)       # 50176
NTOT_Q = float(B * NQ)       # 12544


DEBUG = False
NREP = 1
NO_CC = False


def _build():
    nc = bacc.Bacc("TRN2", target_bir_lowering=False, debug=False,
                   num_devices=N_CORES)
    dt = nc.dram_tensor
    xT_d = dt("xT", [C, R], BF16, kind="ExternalInput").ap()
    wk_d = dt("wkT", [C, KCH], BF16, kind="ExternalInput").ap()
    wv_d = dt("wvT", [C, VCH], BF16, kind="ExternalInput").ap()
    wq_d = dt("wqT", [C, KCH], BF16, kind="ExternalInput").ap()
    wp_d = dt("wpT", [VCH, PCH], BF16, kind="ExternalInput").ap()
    eb_d = dt("ebT", [KC, NKC, H, NQ], BF16, kind="ExternalInput").ap()
    gb_d = dt("gb", [128, 44], F32, kind="ExternalInput").ap()
    y_d = dt("y", [PCH, RQ], F32, kind="ExternalOutput").ap()
    dbg = None
    if DEBUG:
        dbg = {
            "dbg_kh": dt("dbg_kh", [C, R], BF16, kind="ExternalOutput").ap(),
            "dbg_q": dt("dbg_q", [C, RQ], BF16, kind="ExternalOutput").ap(),
            "dbg_ar1": dt("dbg_ar1", [128, 32], F32, kind="ExternalOutput").ap(),
            "dbg_aff": dt("dbg_aff", [128, 26], F32, kind="ExternalOutput").ap(),
            "dbg_hsw": dt("dbg_hsw", [VCH, RQ], BF16, kind="ExternalOutput").ap(),
            "dbg_attn": dt("dbg_attn", [128, NKC, 4, NQ], BF16, kind="ExternalOutput").ap(),
            "dbg_vaug": dt("dbg_vaug", [128, H * (D + 1)], BF16, kind="ExternalOutput").ap(),
            "dbg_av": dt("dbg_av", [D + 1, NQ], F32, kind="ExternalOutput").ap(),
        }

    with tile.TileContext(nc) as tc:
        _kernel(tc, y_d, xT_d, wk_d, wv_d, wq_d, wp_d, eb_d, gb_d, dbg)
    nc.compile()
    return nc


def _kernel(tc, y_d, xT_d, wk_d, wv_d, wq_d, wp_d, eb_d, gb_d, dbg=None):
    nc = tc.nc
    ctx_pools = {}

    # ---------------- persistent SBUF ----------------
    pers = tc.alloc_tile_pool(name="pers", bufs=1)
    gb_sb = pers.tile([128, 44], F32, tag="gb")
    kh_sb = pers.tile([128, 4, R], BF16, tag="kh")       # k^T raw
    q_sb = pers.tile([128, 4, RQ], BF16, tag="qt")       # q^T raw -> q~ after AR1
    hsw_sb = pers.tile([128, 8, RQ], BF16, tag="hsw")    # attnout^T -> hswish
    stat_sb = pers.tile([128, 32], F32, tag="st1")
    ar1_sb = pers.tile([128, 32], F32, tag="ar1")
    stat2_sb = pers.tile([128, 12], F32, tag="st2")
    ar2_sb = pers.tile([128, 12], F32, tag="ar2")
    aff_sb = pers.tile([128, 26], F32, tag="aff")  # qscale4 qbias4 sv8 tv8 sp6? -> see offsets
    # aff cols: 0:4 qscaleA, 4:8 qbiasA, 8:16 svA, 16:24 tvA  (proj affine in aff2)
    aff2_sb = pers.tile([128, 12], F32, tag="aff2")  # 0:6 spA, 6:12 tpA
    eps_sb = pers.tile([128, 1], F32, tag="eps")
    nc.vector.memset(eps_sb[:], EPS)
    dummy_sb = pers.tile([128, RT], F32, tag="dummy")
    nc.vector.memset(dummy_sb[:], 0.0)

    nc.sync.dma_start(gb_sb[:], gb_d[:])

    # DRAM scratch
    dram = tc.alloc_tile_pool(name="dram", bufs=1, space="DRAM")
    vT_dr = dram.tile([VCH, R + 16], BF16, tag="vT")
    yp_dr = dram.tile([PCH, RQ], F32, tag="yp")
    ar1_in = dram.tile([128, 32], F32, tag="ar1i")
    ar1_out = dram.tile([128, 32], F32, tag="ar1o")
    ar2_in = dram.tile([128, 12], F32, tag="ar2i")
    ar2_out = dram.tile([128, 12], F32, tag="ar2o")
    rcp_dr = dram.tile([BL, H, NQ], F32, tag="rcp")

    for _rep in range(NREP):
        _phases(tc, nc, y_d, xT_d, wk_d, wv_d, wq_d, wp_d, eb_d, gb_d, dbg,
                pers, dram, gb_sb, kh_sb, q_sb, hsw_sb, stat_sb, ar1_sb,
                stat2_sb, ar2_sb, aff_sb, aff2_sb, eps_sb, dummy_sb,
                vT_dr, yp_dr, ar1_in, ar1_out, ar2_in, ar2_out, rcp_dr)

    pers.release()
    dram.release()


def _phases(tc, nc, y_d, xT_d, wk_d, wv_d, wq_d, wp_d, eb_d, gb_d, dbg,
            pers, dram, gb_sb, kh_sb, q_sb, hsw_sb, stat_sb, ar1_sb,
            stat2_sb, ar2_sb, aff_sb, aff2_sb, eps_sb, dummy_sb,
            vT_dr, yp_dr, ar1_in, ar1_out, ar2_in, ar2_out, rcp_dr):
    # ================= Phase 1: raw projections + stats =================
    with tc.tile_pool(name="p1sb", bufs=1) as p1sb, \
         tc.tile_pool(name="p1t", bufs=3) as p1t, \
         tc.tile_pool(name="p1ps", bufs=8, space="PSUM") as p1ps:
        wk_sb = p1sb.tile([128, 4, KCH], BF16, tag="wk")
        wv_sb = p1sb.tile([128, 4, VCH], BF16, tag="wv")
        wq_sb = p1sb.tile([128, 4, KCH], BF16, tag="wq")
        for t in range(4):
            nc.sync.dma_start(wk_sb[:, t, :], wk_d.rearrange("(t p) n -> p t n", p=128)[:, t, :])
            nc.sync.dma_start(wv_sb[:, t, :], wv_d.rearrange("(t p) n -> p t n", p=128)[:, t, :])
            nc.sync.dma_start(wq_sb[:, t, :], wq_d.rearrange("(t p) n -> p t n", p=128)[:, t, :])
        xT_sb = p1sb.tile([128, 4, R], BF16, tag="xT")
        for t in range(4):
            nc.sync.dma_start(
                xT_sb[:, t, :],
                xT_d.rearrange("(t p) n -> p t n", p=128)[:, t, :])

        ksum = p1sb.tile([128, 4, NRT], F32, tag="ksum")
        ksq = p1sb.tile([128, 4, NRT], F32, tag="ksq")
        vsum = p1sb.tile([128, 8, NRT], F32, tag="vsum")
        vsq = p1sb.tile([128, 8, NRT], F32, tag="vsq")
        qsum = p1sb.tile([128, 4, NRTQ], F32, tag="qsum")
        qsq = p1sb.tile([128, 4, NRTQ], F32, tag="qsq")

        def proj_tile(dst_slice, w_sb, ct, rt, rhs_fn, sumt, sqt, nrt_cols):
            ps = p1ps.tile([128, RT], F32, tag="ps")
            for cc in range(4):
                nc.tensor.matmul(
                    ps[:], w_sb[:, cc, ct * 128:(ct + 1) * 128], rhs_fn(cc),
                    start=(cc == 0), stop=(cc == 3))
            # fused eviction + per-channel row-sum
            nc.vector.scalar_tensor_tensor(
                out=dst_slice, in0=ps[:], scalar=1.0, in1=dummy_sb[:],
                op0=OP.mult, op1=OP.bypass, accum_out=sumt[:, ct, rt:rt + 1])
            junk = p1t.tile([128, RT], BF16, tag="junk")
            nc.vector.scalar_tensor_tensor(
                out=junk[:], in0=dst_slice, scalar=1.0, in1=dst_slice,
                op0=OP.mult, op1=OP.mult, accum_out=sqt[:, ct, rt:rt + 1])

        # --- v^T (to DRAM) ---
        for ct in range(8):
            for rt in range(NRT):
                ps = p1ps.tile([128, RT], F32, tag="ps")
                for cc in range(4):
                    nc.tensor.matmul(
                        ps[:], wv_sb[:, cc, ct * 128:(ct + 1) * 128],
                        xT_sb[:, cc, rt * RT:(rt + 1) * RT],
                        start=(cc == 0), stop=(cc == 3))
                vtile = p1t.tile([128, RT], BF16, tag="vtile")
                nc.scalar.activation(
                    vtile[:], ps[:], AF.Identity,
                    bias=dummy_sb[:, 0:1], scale=1.0,
                    accum_out=vsum[:, ct, rt:rt + 1])
                junk = p1t.tile([128, RT], BF16, tag="junk")
                nc.vector.scalar_tensor_tensor(
                    out=junk[:], in0=vtile[:], scalar=1.0, in1=vtile[:],
                    op0=OP.mult, op1=OP.mult,
                    accum_out=vsq[:, ct, rt:rt + 1])
                nc.sync.dma_start(
                    vT_dr.rearrange("(t p) n -> p t n", p=128)[:, ct,
                                                              rt * RT:(rt + 1) * RT],
                    vtile[:])

        # --- k^T (stays in SBUF) ---
        for ct in range(4):
            for rt in range(NRT):
                proj_tile(kh_sb[:, ct, rt * RT:(rt + 1) * RT], wk_sb, ct, rt,
                          lambda cc: xT_sb[:, cc, rt * RT:(rt + 1) * RT],
                          ksum, ksq, NRT)

        # --- q^T (subsampled rows; stays in SBUF) ---
        for ct in range(4):
            for rt in range(NRTQ):
                b0 = rt * 2  # 392 q-rows = 2 batches
                def qrhs(cc, b0=b0):
                    base = xT_sb[:, cc, :].rearrange(
                        "p (b i ti j tj) -> p b i ti j tj",
                        b=BL, i=RES_, ti=2, j=RES_, tj=2)
                    return base[:, b0:b0 + 2, :, 0, :, 0]
                proj_tile(q_sb[:, ct, rt * RT:(rt + 1) * RT], wq_sb, ct, rt,
                          qrhs, qsum, qsq, NRTQ)

        # --- fold stats into AR block ---
        nc.vector.tensor_reduce(stat_sb[:, 0:4], ksum[:], axis=mybir.AxisListType.X, op=OP.add)
        nc.vector.tensor_reduce(stat_sb[:, 4:8], ksq[:], axis=mybir.AxisListType.X, op=OP.add)
        nc.vector.tensor_reduce(stat_sb[:, 8:12], qsum[:], axis=mybir.AxisListType.X, op=OP.add)
        nc.vector.tensor_reduce(stat_sb[:, 12:16], qsq[:], axis=mybir.AxisListType.X, op=OP.add)
        nc.vector.tensor_reduce(stat_sb[:, 16:24], vsum[:], axis=mybir.AxisListType.X, op=OP.add)
        nc.vector.tensor_reduce(stat_sb[:, 24:32], vsq[:], axis=mybir.AxisListType.X, op=OP.add)

    # ---------------- AllReduce #1 ----------------
    nc.sync.dma_start(ar1_in[:], stat_sb[:])
    if NO_CC:
        nc.sync.dma_start(ar1_out[:], ar1_in[:])
    else:
        nc.gpsimd.collective_compute(
            "AllReduce", OP.add, replica_groups=[list(range(N_CORES))],
            ins=[ar1_in.opt()], outs=[ar1_out.opt()])
    nc.sync.dma_start(ar1_sb[:], ar1_out[:])

    # ---------------- post-AR1 affine math (tiny) ----------------
    with tc.tile_pool(name="aff_t", bufs=1) as afft:
        tmp = afft.tile([128, 16], F32, tag="t0")
        # k: cols 0:4 mean, 4:8 E[x^2]
        mk = tmp[:, 0:4]
        nc.vector.tensor_scalar_mul(mk, ar1_sb[:, 0:4], 1.0 / NTOT_KV)
        vk = tmp[:, 4:8]
        nc.vector.tensor_scalar_mul(vk, ar1_sb[:, 4:8], 1.0 / NTOT_KV)
        mk2 = tmp[:, 8:12]
        nc.vector.tensor_mul(mk2, mk, mk)
        nc.vector.tensor_sub(vk, vk, mk2)
        sdk = tmp[:, 8:12]
        nc.scalar.activation(sdk, vk, AF.Sqrt, bias=eps_sb[:])
        rsk = tmp[:, 4:8]
        nc.vector.reciprocal(rsk, sdk)
        skf = tmp[:, 8:12]  # full k scale = g_k / sd
        nc.vector.tensor_mul(skf, gb_sb[:, 0:4], rsk)

        # q: ar cols 8:12 sum, 12:16 sumsq ; gb cols 8:12 gq, 12:16 bq
        tq = afft.tile([128, 16], F32, tag="t1")
        mq = tq[:, 0:4]
        nc.vector.tensor_scalar_mul(mq, ar1_sb[:, 8:12], 1.0 / NTOT_Q)
        vq = tq[:, 4:8]
        nc.vector.tensor_scalar_mul(vq, ar1_sb[:, 12:16], 1.0 / NTOT_Q)
        mq2 = tq[:, 8:12]
        nc.vector.tensor_mul(mq2, mq, mq)
        nc.vector.tensor_sub(vq, vq, mq2)
        sdq = tq[:, 8:12]
        nc.scalar.activation(sdq, vq, AF.Sqrt, bias=eps_sb[:])
        rsq = tq[:, 4:8]
        nc.vector.reciprocal(rsq, sdq)
        sqf = tq[:, 8:12]
        nc.vector.tensor_mul(sqf, gb_sb[:, 8:12], rsq)
        # qscaleA = sqf * skf * SCALE ; qbiasA = (bq - mq*sqf) * skf * SCALE
        nc.vector.tensor_mul(aff_sb[:, 0:4], sqf, skf)
        nc.vector.tensor_scalar_mul(aff_sb[:, 0:4], aff_sb[:, 0:4], SCALE)
        tq2 = tq[:, 12:16]
        nc.vector.tensor_mul(tq2, mq, sqf)
        nc.vector.tensor_sub(tq2, gb_sb[:, 12:16], tq2)
        nc.vector.tensor_mul(aff_sb[:, 4:8], tq2, skf)
        nc.vector.tensor_scalar_mul(aff_sb[:, 4:8], aff_sb[:, 4:8], SCALE)

        # v: ar cols 16:24 sum, 24:32 sumsq; gb 16:24 gv, 24:32 bv
        tv = afft.tile([128, 24], F32, tag="t2")
        mv = tv[:, 0:8]
        nc.vector.tensor_scalar_mul(mv, ar1_sb[:, 16:24], 1.0 / NTOT_KV)
        vv = tv[:, 8:16]
        nc.vector.tensor_scalar_mul(vv, ar1_sb[:, 24:32], 1.0 / NTOT_KV)
        mv2 = tv[:, 16:24]
        nc.vector.tensor_mul(mv2, mv, mv)
        nc.vector.tensor_sub(vv, vv, mv2)
        sdv = tv[:, 16:24]
        nc.scalar.activation(sdv, vv, AF.Sqrt, bias=eps_sb[:])
        rsv = tv[:, 8:16]
        nc.vector.reciprocal(rsv, sdv)
        nc.vector.tensor_mul(aff_sb[:, 8:16], gb_sb[:, 16:24], rsv)   # svA
        nc.vector.tensor_mul(tv[:, 16:24], mv, aff_sb[:, 8:16])
        nc.vector.tensor_sub(aff_sb[:, 16:24], gb_sb[:, 24:32], tv[:, 16:24])  # tvA

    # q~ = q_raw * qscaleA + qbiasA (in place)
    for ct in range(4):
        nc.scalar.activation(q_sb[:, ct, :], q_sb[:, ct, :], AF.Identity,
                             bias=aff_sb[:, 4 + ct:5 + ct],
                             scale=aff_sb[:, ct:ct + 1])

    # ================= Phase 2: attention =================
    with tc.tile_pool(name="ebp", bufs=1) as ebp, \
         tc.tile_pool(name="vstream", bufs=8) as vstream, \
         tc.tile_pool(name="attnp", bufs=3) as attnp, \
         tc.tile_pool(name="smallp", bufs=4) as smallp, \
         tc.tile_pool(name="qblkp", bufs=2) as qblkp, \
         tc.tile_pool(name="qkps", bufs=2, space="PSUM") as qkps, \
         tc.tile_pool(name="avps", bufs=4, space="PSUM") as avps:
        eb_sb = ebp.tile([128, NKC, H, NQ], BF16, tag="eb")
        nc.sync.dma_start(eb_sb[0:KC, :, :, :], eb_d[:])

        for b in range(BL):
            vts = []
            for kc in range(NKC):
                vtmp = vstream.tile([128, VCH], BF16, tag="vtmp")
                nc.sync.dma_start_transpose(
                    vtmp[:], vT_dr[:, b * N + kc * KC: b * N + kc * KC + 128])
                vt = vstream.tile([128, H * (D + 1)], BF16, tag="vaug")
                nc.gpsimd.tensor_copy(
                    vt.rearrange("p (h d) -> p h d", h=H)[:, :, 0:D],
                    vtmp.rearrange("p (h d) -> p h d", h=H)[:, :, :])
                nc.vector.memset(
                    vt.rearrange("p (h d) -> p h d", h=H)[0:KC, :, D:D + 1],
                    1.0)
                if dbg is not None and b == 0 and kc == 0:
                    nc.sync.dma_start(dbg["dbg_vaug"][:], vt[:])
                vts.append(vt)

            for quad in range(4):
                attn = attnp.tile([128, NKC, 4, NQ], BF16, tag="attn")
                # block-sparse packed q: rows hq*32..+32 hold head hq's q at
                # cols hq*196..+196, zeros elsewhere -> one [128,112] LDW per
                # (quad, kc) instead of four, K=128 contraction.
                qblk = qblkp.tile([128, 4, NQ], BF16, tag="qblk")
                nc.vector.memset(qblk[:], 0.0)
                for hq in range(4):
                    nc.gpsimd.tensor_copy(
                        qblk[hq * 32:(hq + 1) * 32, hq, :],
                        q_sb[hq * 32:(hq + 1) * 32, quad,
                             b * NQ:(b + 1) * NQ])
                for kc in range(NKC):
                    qk = qkps.tile([128, 2, 512], F32, tag="qk")
                    for j in range(2):
                        nc.tensor.matmul(
                            qk[0:KC, j, 0:2 * NQ],
                            kh_sb[:, quad,
                                  b * N + kc * KC: b * N + (kc + 1) * KC],
                            qblk[:, 2 * j:2 * j + 2, :],
                            start=True, stop=True)
                    attn_flat = attn[:].rearrange("p a b c -> p a (b c)")
                    nc.scalar.activation(
                        attn_flat[0:KC, kc, :].rearrange(
                            "p (j x) -> p j x", j=2),
                        qk[0:KC, :, 0:2 * NQ], AF.Exp)
                # multiply by exp(rel-pos bias), in place
                nc.vector.tensor_mul(
                    attn[0:KC, :, :, :],
                    attn[0:KC, :, :, :],
                    eb_sb[0:KC, :, quad * 4:(quad + 1) * 4, :])
                if dbg is not None and b == 0 and quad == 0:
                    nc.sync.dma_start(dbg["dbg_attn"][:], attn[:])

                for hq in range(4):
                    h = quad * 4 + hq
                    av = avps.tile([D + 1, NQ], F32, tag="av")
                    for kc in range(NKC):
                        nc.tensor.matmul(
                            av[:], vts[kc][0:KC, h * (D + 1):(h + 1) * (D + 1)],
                            attn[0:KC, kc, hq, :],
                            start=(kc == 0), stop=(kc == NKC - 1))
                    if dbg is not None and b == 0 and quad == 0 and hq == 0:
                        avdbg = smallp.tile([D + 1, NQ], F32, tag="avdbg")
                        nc.vector.tensor_copy(avdbg[:], av[:])
                        nc.sync.dma_start(dbg["dbg_av"][:], avdbg[:])
                    rrow = smallp.tile([1, NQ], F32, tag="rrow")
                    nc.vector.reciprocal(rrow[:], av[D:D + 1, :])
                    nc.sync.dma_start(rcp_dr[b, h, :], rrow[:])
                    rbc = smallp.tile([64, NQ], F32, tag="rbc")
                    rap = rcp_dr[b, h, :]
                    nc.gpsimd.dma_start(
                        rbc[:],
                        bass.AP(tensor=rap.tensor, offset=rap.offset,
                                ap=[[0, 64]] + list(rap.ap)))
                    nc.vector.tensor_mul(
                        hsw_sb[(h % 2) * 64:(h % 2) * 64 + 64, h // 2,
                               b * NQ:(b + 1) * NQ],
                        av[0:D, :], rbc[:])

    # ---------------- hard-swish (with v affine) ----------------
    with tc.tile_pool(name="hswp", bufs=2) as hswp:
        for t in range(8):
            u = hsw_sb[:, t, :]
            nc.scalar.activation(u, u, AF.Identity,
                                 bias=aff_sb[:, 16 + t:17 + t],
                                 scale=aff_sb[:, 8 + t:9 + t])
            z = hswp.tile([128, RQ], BF16, tag="z")
            nc.vector.tensor_scalar(out=z[:], in0=u, scalar1=3.0, scalar2=0.0,
                                    op0=OP.add, op1=OP.max)
            nc.vector.tensor_scalar(out=z[:], in0=z[:], scalar1=6.0,
                                    scalar2=1.0 / 6.0, op0=OP.min, op1=OP.mult)
            nc.vector.tensor_mul(u, u, z[:])

    if dbg is not None:
        for t in range(4):
            nc.sync.dma_start(
                dbg["dbg_kh"].rearrange("(t p) n -> p t n", p=128)[:, t, :],
                kh_sb[:, t, :])
            nc.sync.dma_start(
                dbg["dbg_q"].rearrange("(t p) n -> p t n", p=128)[:, t, :],
                q_sb[:, t, :])
        for t in range(8):
            nc.sync.dma_start(
                dbg["dbg_hsw"].rearrange("(t p) n -> p t n", p=128)[:, t, :],
                hsw_sb[:, t, :])
        nc.sync.dma_start(dbg["dbg_ar1"][:], ar1_sb[:])
        nc.sync.dma_start(dbg["dbg_aff"][:], aff_sb[:])

    # ================= Phase 3: proj + BN =================
    with tc.tile_pool(name="p3w", bufs=1) as p3w, \
         tc.tile_pool(name="p3t", bufs=4) as p3t, \
         tc.tile_pool(name="p3ps", bufs=4, space="PSUM") as p3ps:
        wp_sb = p3w.tile([128, 8, PCH], BF16, tag="wp")
        for t in range(8):
            nc.sync.dma_start(wp_sb[:, t, :], wp_d.rearrange("(t p) n -> p t n", p=128)[:, t, :])
        psum_acc = p3t.tile([128, 6, NRTQ], F32, tag="psum_acc")
        psq_acc = p3t.tile([128, 6, NRTQ], F32, tag="psq_acc")
        for pt in range(6):
            for rt in range(NRTQ):
                ps = p3ps.tile([128, RT], F32, tag="pps")
                for cc in range(8):
                    nc.tensor.matmul(
                        ps[:], wp_sb[:, cc, pt * 128:(pt + 1) * 128],
                        hsw_sb[:, cc, rt * RT:(rt + 1) * RT],
                        start=(cc == 0), stop=(cc == 7))
                yb = p3t.tile([128, RT], F32, tag="yb")
                nc.vector.scalar_tensor_tensor(
                    out=yb[:], in0=ps[:], scalar=1.0, in1=dummy_sb[:],
                    op0=OP.mult, op1=OP.bypass,
                    accum_out=psum_acc[:, pt, rt:rt + 1])
                junk = p3t.tile([128, RT], F32, tag="junk3")
                nc.vector.scalar_tensor_tensor(
                    out=junk[:], in0=yb[:], scalar=1.0, in1=yb[:],
                    op0=OP.mult, op1=OP.mult,
                    accum_out=psq_acc[:, pt, rt:rt + 1])
                nc.sync.dma_start(
                    yp_dr.rearrange("(t p) n -> p t n", p=128)[:, pt,
                                                               rt * RT:(rt + 1) * RT],
                    yb[:])
        nc.vector.tensor_reduce(stat2_sb[:, 0:6], psum_acc[:],
                                axis=mybir.AxisListType.X, op=OP.add)
        nc.vector.tensor_reduce(stat2_sb[:, 6:12], psq_acc[:],
                                axis=mybir.AxisListType.X, op=OP.add)

    nc.sync.dma_start(ar2_in[:], stat2_sb[:])
    if NO_CC:
        nc.sync.dma_start(ar2_out[:], ar2_in[:])
    else:
        nc.gpsimd.collective_compute(
            "AllReduce", OP.add, replica_groups=[list(range(N_CORES))],
            ins=[ar2_in.opt()], outs=[ar2_out.opt()])
    nc.sync.dma_start(ar2_sb[:], ar2_out[:])

    with tc.tile_pool(name="finp", bufs=3) as finp, \
         tc.tile_pool(name="fint", bufs=1) as fint:
        tp = fint.tile([128, 18], F32, tag="tp")
        mp = tp[:, 0:6]
        nc.vector.tensor_scalar_mul(mp, ar2_sb[:, 0:6], 1.0 / NTOT_Q)
        vp = tp[:, 6:12]
        nc.vector.tensor_scalar_mul(vp, ar2_sb[:, 6:12], 1.0 / NTOT_Q)
        mp2 = tp[:, 12:18]
        nc.vector.tensor_mul(mp2, mp, mp)
        nc.vector.tensor_sub(vp, vp, mp2)
        sdp = tp[:, 12:18]
        nc.scalar.activation(sdp, vp, AF.Sqrt, bias=eps_sb[:])
        rsp = tp[:, 6:12]
        nc.vector.reciprocal(rsp, sdp)
        nc.vector.tensor_mul(aff2_sb[:, 0:6], gb_sb[:, 32:38], rsp)      # spA
        nc.vector.tensor_mul(tp[:, 12:18], mp, aff2_sb[:, 0:6])
        nc.vector.tensor_sub(aff2_sb[:, 6:12], gb_sb[:, 38:44], tp[:, 12:18])  # tpA

        for pt in range(6):
            yt = finp.tile([128, RQ], F32, tag="yt")
            nc.sync.dma_start(
                yt[:], yp_dr.rearrange("(t p) n -> p t n", p=128)[:, pt, :])
            nc.scalar.activation(yt[:], yt[:], AF.Identity,
                                 bias=aff2_sb[:, 6 + pt:7 + pt],
                                 scale=aff2_sb[:, pt:pt + 1])
            nc.sync.dma_start(
                y_d.rearrange("(t p) n -> p t n", p=128)[:, pt, :], yt[:])


# ==================== host staging ====================

_K_IDX = np.array([h * (KD + D) + j for h in range(H) for j in range(KD)])
_V_IDX = np.array([h * (KD + D) + KD + j for h in range(H) for j in range(D)])


def _stage(inputs):
    """Full inputs -> (shared_map, list of per-core xT)."""
    x = np.asarray(inputs["x"], np.float32)
    W_kv = np.asarray(inputs["W_kv"], np.float32)
    g_kv = np.asarray(inputs["g_kv"], np.float32)
    b_kv = np.asarray(inputs["b_kv"], np.float32)
    W_q = np.asarray(inputs["W_q"], np.float32)
    W_p = np.asarray(inputs["W_proj"], np.float32)
    ab = np.asarray(inputs["attn_biases"], np.float32)
    bi = np.asarray(inputs["bias_idxs"])

    shared = {}
    shared["wkT"] = np.ascontiguousarray(W_kv[_K_IDX].T).astype(BF16NP)
    shared["wvT"] = np.ascontiguousarray(W_kv[_V_IDX].T).astype(BF16NP)
    shared["wqT"] = np.ascontiguousarray(W_q.T).astype(BF16NP)
    shared["wpT"] = np.ascontiguousarray(W_p.T).astype(BF16NP)

    eb = np.exp(ab[:, bi])                     # [16, 196, 784]
    ebT = eb.transpose(0, 2, 1).reshape(H, NKC, KC, NQ).transpose(2, 1, 0, 3)
    shared["ebT"] = np.ascontiguousarray(ebT).astype(BF16NP)

    gb = np.zeros((128, 44), np.float32)
    gb[:, 0:4] = g_kv[_K_IDX].reshape(4, 128).T
    gb[:, 4:8] = b_kv[_K_IDX].reshape(4, 128).T
    gb[:, 8:12] = np.asarray(inputs["g_q"], np.float32).reshape(4, 128).T
    gb[:, 12:16] = np.asarray(inputs["b_q"], np.float32).reshape(4, 128).T
    gb[:, 16:24] = g_kv[_V_IDX].reshape(8, 128).T
    gb[:, 24:32] = b_kv[_V_IDX].reshape(8, 128).T
    gb[:, 32:38] = np.asarray(inputs["g_proj"], np.float32).reshape(6, 128).T
    gb[:, 38:44] = np.asarray(inputs["b_proj"], np.float32).reshape(6, 128).T
    shared["gb"] = gb

    xts = []
    for c in range(N_CORES):
        xl = x[c * BL:(c + 1) * BL]                      # [8, 784, 512]
        xts.append(np.ascontiguousarray(
            xl.transpose(2, 0, 1).reshape(C, R)).astype(BF16NP))
    return shared, xts


_nc = None


def _get_nc():
    global _nc
    if _nc is None:
        _nc = _build()
    return _nc


def kernel(**inputs):
    import jax
    dargs = _device_args(inputs)
    outs = run_on_device(dargs)
    jax.block_until_ready(outs)
    _, in_names, out_names, out_avals, _ = _get_jit()
    yi = out_names.index("y")
    yp_all = np.asarray(outs[yi]).reshape(N_CORES, PCH, RQ)
    out = np.empty((B, NQ, PCH), np.float32)
    for c in range(N_CORES):
        out[c * BL:(c + 1) * BL] = yp_all[c].T.reshape(BL, NQ, PCH)
    return out


# -------- device-resident timing protocol (mirrors previous baseline) --------

_jit_state = None


def _get_jit():
    """Build (once) a cached jitted shard_map executor for the NEFF."""
    global _jit_state
    if _jit_state is not None:
        return _jit_state
    import jax
    from jax.sharding import Mesh, PartitionSpec
    from jax.experimental.shard_map import shard_map
    from concourse import bass2jax, mybir as _mb

    nc = _get_nc()
    bass2jax.install_neuronx_cc_hook()
    partition_name = (nc.partition_id_tensor.name
                      if nc.partition_id_tensor else None)
    in_names, out_names, out_avals = [], [], []
    for alloc in nc.m.functions[0].allocations:
        if not isinstance(alloc, _mb.MemoryLocationSet):
            continue
        name = alloc.memorylocations[0].name
        if alloc.kind == "ExternalInput":
            if name != partition_name:
                in_names.append(name)
        elif alloc.kind == "ExternalOutput":
            out_names.append(name)
            out_avals.append(jax.core.ShapedArray(
                tuple(alloc.tensor_shape), _mb.dt.np(alloc.dtype)))
    n_params = len(in_names)
    all_in = in_names + out_names
    if partition_name is not None:
        all_in = all_in + [partition_name]

    def _body(*args):
        operands = list(args)
        if partition_name is not None:
            operands.append(bass2jax.partition_id_tensor())
        outs = bass2jax._bass_exec_p.bind(
            *operands, out_avals=tuple(out_avals),
            in_names=tuple(all_in), out_names=tuple(out_names),
            lowering_input_output_aliases=(),
            sim_require_finite=True, sim_require_nnan=True, nc=nc)
        return tuple(outs)

    devices = jax.devices()[:N_CORES]
    mesh = Mesh(np.asarray(devices), ("core",))
    n_outs = len(out_names)
    sharded = jax.jit(shard_map(
        _body, mesh=mesh,
        in_specs=(PartitionSpec("core"),) * (n_params + n_outs),
        out_specs=(PartitionSpec("core"),) * n_outs,
        check_rep=False), keep_unused=True)
    _jit_state = (sharded, in_names, out_names, out_avals, mesh)
    return _jit_state


def _device_args(inputs):
    import jax
    from jax.sharding import NamedSharding, PartitionSpec
    sharded, in_names, out_names, out_avals, mesh = _get_jit()
    sh = NamedSharding(mesh, PartitionSpec("core"))
    shared, xts = _stage(inputs)
    per_core = [{**shared, "xT": xts[c]} for c in range(N_CORES)]
    concat = [np.concatenate([np.asarray(per_core[c][n])
                              for c in range(N_CORES)], axis=0)
              for n in in_names]
    zeros = [np.zeros((N_CORES * a.shape[0], *a.shape[1:]), a.dtype)
             for a in out_avals]
    return tuple(jax.device_put(a, sh) for a in (*concat, *zeros))


def run_on_device(dargs):
    sharded, *_ = _get_jit()
    return sharded(*dargs)


if __name__ == "__main__":
    import reference
    inputs = {k: np.asarray(v) for k, v in reference.setup_inputs().items()}
    expected = np.asarray(reference.reference(**inputs))
    actual = kernel(**inputs)
    err = np.linalg.norm(actual - expected) / np.linalg.norm(expected)
    print("Relative error:", err)
